# revision 1
# baseline (speedup 1.0000x reference)
"""Trainium2 Bass kernel for nn_MultiHeadAttention (B=2048, T=32, E=1024, H=16).

Sharding: data-parallel over batch, 256 batches per core x 8 cores.
Per-core pipeline (all matmuls fp16 with fp32 PSUM accumulation):
  XT  = X^T                      (PE transpose, fp32 -> fp16 on evac)
  QT  = (Wq/8)^T @ ... = [hd,bt] (stationary = Wq chunk, moving = XT)
  KT  = same for Wk
  V'  = [bt, hd] with a ones column per head (softmax denominator trick)
  per (head, 4-batch group):
    S  = KT_slice.T @ QT_slice   -> [128,128] all cross-batch scores
    EB = exp(S) * maskEB         (block-diag causal mask kills cross terms)
    U' = EB.T @ V'_slice         -> [t, 64+1]; col 64 = softmax denominator
    O  = U'[:, :64] * 1/U'[:,64] (fused into PSUM evac copies)
  OT  = O^T                      (PE transpose)
  y   = OT.T @ Wp + bp           (bias via K=1 ones-row matmul)
"""
import sys
import os
import numpy as np

sys.path.insert(0, "/opt/trn_rl_repo")

import concourse.bass as bass  # noqa: E402
import concourse.bacc as bacc  # noqa: E402
import concourse.mybir as mybir  # noqa: E402
import concourse.tile as tile  # noqa: E402
import contextlib  # noqa: E402
from concourse.bass_utils import run_bass_kernel_spmd  # noqa: E402

B, T, E, H = 2048, 32, 1024, 16
HS = E // H            # 64
NCORES = 8
BC = B // NCORES       # 256 batches per core
BT = BC * T            # 8192 rows per core
P = 128
ET = E // P            # 8 e-tiles
MT = 512               # rows per macro-tile
NMT = BT // MT         # 16
NBT = MT // P          # 4 bt-tiles (= 4-batch groups) per macro-tile

F16 = mybir.dt.float16
F32 = mybir.dt.float32
AF = mybir.ActivationFunctionType

_CACHE = {}


def _build_nc(nmt=NMT, stage=7, repeat=1):
    bt = nmt * MT
    nc = bacc.Bacc(trn_type="TRN2")

    x_d = nc.dram_tensor("xs", [bt, E], F32, kind="ExternalInput")
    y_d = nc.dram_tensor("ys", [bt, E], F32, kind="ExternalOutput")
    wq_d = nc.dram_tensor("wq", [P, ET * E], F16, kind="ExternalInput")
    wk_d = nc.dram_tensor("wk", [P, ET * E], F16, kind="ExternalInput")
    wv_d = nc.dram_tensor("wv", [P, ET * E], F16, kind="ExternalInput")
    wp_d = nc.dram_tensor("wp", [P, ET * E], F16, kind="ExternalInput")
    bp_d = nc.dram_tensor("bp", [1, E], F16, kind="ExternalInput")
    mask_d = nc.dram_tensor("mask", [P, 512], F16, kind="ExternalInput")
    id32_d = nc.dram_tensor("id32", [P, P], F32, kind="ExternalInput")
    id16_d = nc.dram_tensor("id16", [P, P], F16, kind="ExternalInput")

    with tile.TileContext(nc) as tc:
        with (
            tc.tile_pool(name="const", bufs=1) as cpool,
            tc.tile_pool(name="xin", bufs=6) as xpool,
            tc.tile_pool(name="xt", bufs=12) as xtpool,
            tc.tile_pool(name="qt", bufs=12) as qtpool,
            tc.tile_pool(name="kt", bufs=12) as ktpool,
            tc.tile_pool(name="vp", bufs=6) as vppool,
            tc.tile_pool(name="eb", bufs=8) as ebpool,
            tc.tile_pool(name="osb", bufs=6) as opool,
            tc.tile_pool(name="ot", bufs=12) as otpool,
            tc.tile_pool(name="rc", bufs=8) as rcpool,
            tc.tile_pool(name="yo", bufs=8) as ypool,
            tc.tile_pool(name="ps_mm", bufs=3, space="PSUM") as psmm,
            tc.tile_pool(name="ps_tr", bufs=3, space="PSUM") as pstr,
            tc.tile_pool(name="ps_u", bufs=2, space="PSUM") as psu,
        ):
            wq = cpool.tile([P, ET * E], F16)
            wk = cpool.tile([P, ET * E], F16)
            wv = cpool.tile([P, ET * E], F16)
            wp = cpool.tile([P, ET * E], F16)
            bpt = cpool.tile([1, E], F16)
            mask = cpool.tile([P, 512], F16)
            id32 = cpool.tile([P, P], F32)
            id16 = cpool.tile([P, P], F16)
            ones1 = cpool.tile([1, P], F16)

            nc.sync.dma_start(wq[:], wq_d[:])
            nc.sync.dma_start(wk[:], wk_d[:])
            nc.sync.dma_start(wv[:], wv_d[:])
            nc.sync.dma_start(wp[:], wp_d[:])
            nc.sync.dma_start(bpt[:], bp_d[:])
            nc.sync.dma_start(mask[:], mask_d[:])
            nc.sync.dma_start(id32[:], id32_d[:])
            nc.sync.dma_start(id16[:], id16_d[:])
            nc.vector.memset(ones1[:], 1.0)

            x_v = x_d.rearrange("(m b p) e -> m b p e", b=NBT, p=P)
            y_v = y_d.rearrange("(m b p) e -> m b p e", b=NBT, p=P)

            rep_cm = tc.For_i(0, repeat) if repeat > 1 else contextlib.nullcontext()
            with rep_cm:
              for mt in range(nmt):
                  # ---- load X ----
                  xb = []
                  for b in range(NBT):
                      xt_in = xpool.tile([P, E], F32, tag="x")
                      nc.sync.dma_start(xt_in[:], x_v[mt, b])
                      xb.append(xt_in)


                  def _dump(t16):
                      dmp = ypool.tile([P, 512], F32, tag="y")
                      nc.vector.tensor_copy(dmp[:], t16[:, 0:512])
                      nc.sync.dma_start(y_v[mt, 0][:, 0:512], dmp[:])

                  # ---- transpose X -> XT (fp16) ----
                  xts = []
                  for et in range(ET):
                      pt = pstr.tile([P, 512], F32, tag="ps_tr")
                      for b in range(NBT):
                          nc.tensor.transpose(
                              pt[:, P * b:P * (b + 1)],
                              xb[b][:, P * et:P * (et + 1)],
                              id32[:],
                          )
                      xt_t = xtpool.tile([P, 512], F16, tag="xt")
                      nc.scalar.activation(xt_t[:], pt[:], AF.Copy)
                      xts.append(xt_t)

                  if stage <= 1:
                      _dump(xts[0])
                      continue

                  # ---- QT / KT projections: [hd, bt] ----
                  qts, kts = [], []
                  for w_sb, pool, outl in ((wq, qtpool, qts), (wk, ktpool, kts)):
                      for ht in range(ET):
                          pq = psmm.tile([P, 512], F32, tag="ps_mm")
                          for et in range(ET):
                              nc.tensor.matmul(
                                  pq[:],
                                  w_sb[:, et * E + P * ht:et * E + P * (ht + 1)],
                                  xts[et][:],
                                  start=(et == 0),
                                  stop=(et == ET - 1),
                              )
                          sb = pool.tile([P, 512], F16)
                          nc.vector.tensor_copy(sb[:], pq[:])
                          outl.append(sb)

                  if stage <= 2:
                      _dump(qts[0])
                      continue

                  # ---- V projection -> V' [bt, 16*(64+1)] with ones cols ----
                  vps = []
                  for b in range(NBT):
                      vp_t = vppool.tile([P, H * (HS + 1)], F16, tag="vp")
                      nc.vector.memset(
                          vp_t.rearrange("p (h c) -> p h c", c=HS + 1)[:, :, HS:HS + 1],
                          1.0,
                      )
                      for h2 in range(2):
                          pv = psmm.tile([P, 512], F32, tag="ps_mm")
                          for et in range(ET):
                              nc.tensor.matmul(
                                  pv[:],
                                  xts[et][:, P * b:P * (b + 1)],
                                  wv[:, et * E + 512 * h2:et * E + 512 * (h2 + 1)],
                                  start=(et == 0),
                                  stop=(et == ET - 1),
                              )
                          dst = vp_t[:, 8 * (HS + 1) * h2:8 * (HS + 1) * (h2 + 1)]
                          nc.vector.tensor_copy(
                              dst.rearrange("p (h c) -> p h c", c=HS + 1)[:, :, 0:HS],
                              pv.rearrange("p (h c) -> p h c", c=HS)[:],
                          )
                      vps.append(vp_t)

                  if stage <= 3:
                      _dump(vps[0])
                      continue

                  # ---- attention ----
                  os_ = []
                  for b in range(NBT):
                      if not (4.0 < stage < 4.5):
                          o_t = opool.tile([P, E], F16, tag="o")
                          os_.append(o_t)
                      HEAD_GROUPS = ((0, 2, 4, 6), (8, 10, 12, 14),
                                     (1, 3, 5, 7), (9, 11, 13, 15))
                      for hq in range(4):
                          heads = HEAD_GROUPS[hq]
                          ps_s = pstr.tile([P, 512], F32, tag="ps_tr")
                          for hh in range(4):
                              h = heads[hh]
                              ht, hp = divmod(h, 2)
                              rs = slice(64 * hp, 64 * (hp + 1))
                              cs = slice(P * b, P * (b + 1))
                              nc.tensor.matmul(
                                  ps_s[:, P * hh:P * (hh + 1)],
                                  kts[ht][rs, cs],
                                  qts[ht][rs, cs],
                                  start=True,
                                  stop=True,
                              )
                          if stage <= 4.1:
                              ebt = ebpool.tile([P, 512], F16, tag="eb")
                              nc.vector.tensor_copy(ebt[:], ps_s[:])
                              _dump(ebt)
                              continue
                          ebt = ebpool.tile([P, 512], F16, tag="eb")
                          nc.scalar.activation(ebt[:], ps_s[:], AF.Exp)
                          if stage <= 4.2:
                              _dump(ebt)
                              continue
                          nc.vector.tensor_mul(ebt[:], ebt[:], mask[:])
                          if stage <= 4.3:
                              _dump(ebt)
                              continue
                          pu = psu.tile([P, 4 * (HS + 1)], F32, tag="ps_u")
                          for hh in range(4):
                              h = heads[hh]
                              nc.tensor.matmul(
                                  pu[:, (HS + 1) * hh:(HS + 1) * (hh + 1)],
                                  ebt[:, P * hh:P * (hh + 1)],
                                  vps[b][:, (HS + 1) * h:(HS + 1) * (h + 1)],
                                  start=True,
                                  stop=True,
                              )
                          if stage <= 4.4:
                              u16 = ebpool.tile([P, 512], F16, tag="eb")
                              nc.vector.tensor_copy(u16[:, 0:260], pu[:])
                              _dump(u16)
                              continue
                          rc_t = rcpool.tile([P, 4], F32, tag="rc")
                          nc.vector.reciprocal(
                              rc_t[:],
                              pu.rearrange("p (h c) -> p h c", c=HS + 1)[:, :, HS:HS + 1],
                          )
                          for hh in range(4):
                              h = heads[hh]
                              dst = o_t[:, HS * h:HS * (h + 1)]
                              src = pu[:, (HS + 1) * hh:(HS + 1) * hh + HS]
                              sc = rc_t[:, hh:hh + 1]
                              if hh % 2 == 0:
                                  nc.scalar.activation(dst, src, AF.Copy, scale=sc)
                              else:
                                  nc.vector.tensor_scalar_mul(dst, src, sc)

                  if 4.0 < stage < 4.5:
                      continue
                  if stage <= 5:
                      if os_:
                          _dump(os_[0])
                      continue

                  # ---- transpose O -> OT ----
                  ots = []
                  for ht in range(ET):
                      pt = pstr.tile([P, 512], F16, tag="ps_tr")
                      for b in range(NBT):
                          nc.tensor.transpose(
                              pt[:, P * b:P * (b + 1)],
                              os_[b][:, P * ht:P * (ht + 1)],
                              id16[:],
                          )
                      ot_t = otpool.tile([P, 512], F16, tag="ot")
                      nc.scalar.activation(ot_t[:], pt[:], AF.Copy)
                      ots.append(ot_t)

                  if stage <= 6:
                      _dump(ots[0])
                      continue

                  # ---- output projection + bias ----
                  for b in range(NBT):
                      for e2 in range(2):
                          py = psmm.tile([P, 512], F32, tag="ps_mm")
                          nc.tensor.matmul(
                              py[:],
                              ones1[:],
                              bpt[:, 512 * e2:512 * (e2 + 1)],
                              start=True,
                              stop=False,
                          )
                          for ht in range(ET):
                              nc.tensor.matmul(
                                  py[:],
                                  ots[ht][:, P * b:P * (b + 1)],
                                  wp[:, ht * E + 512 * e2:ht * E + 512 * (e2 + 1)],
                                  start=False,
                                  stop=(ht == ET - 1),
                              )
                          y_t = ypool.tile([P, 512], F32, tag="y")
                          if e2 == 0:
                              nc.scalar.activation(y_t[:], py[:], AF.Copy)
                          else:
                              nc.vector.tensor_copy(y_t[:], py[:])
                          nc.sync.dma_start(
                              y_v[mt, b][:, 512 * e2:512 * (e2 + 1)], y_t[:]
                          )

    nc.compile()
    return nc


def _host_prep(Wq, Wk, Wv, Wp, bp):
    def cat(w):  # [H, E, HS] -> [E, E]
        return np.ascontiguousarray(w.transpose(1, 0, 2).reshape(E, E))

    def sb_layout(w16):  # [E, E] f16 -> [128, 8*E]
        return np.ascontiguousarray(
            w16.reshape(ET, P, E).transpose(1, 0, 2).reshape(P, ET * E)
        )

    wq16 = sb_layout((cat(Wq) * (HS ** -0.5)).astype(np.float16))
    wk16 = sb_layout(cat(Wk).astype(np.float16))
    wv16 = sb_layout(cat(Wv).astype(np.float16))
    wp16 = sb_layout(Wp.astype(np.float16))
    bp16 = bp.astype(np.float16).reshape(1, E)

    m = np.zeros((P, P), dtype=np.float16)
    trilT = np.tril(np.ones((T, T))).T.astype(np.float16)  # [s,t], s<=t
    for i in range(4):
        m[T * i:T * (i + 1), T * i:T * (i + 1)] = trilT
    mask = np.ascontiguousarray(np.tile(m, (1, 4)))

    id32 = np.eye(P, dtype=np.float32)
    id16 = np.eye(P, dtype=np.float16)
    return dict(wq=wq16, wk=wk16, wv=wv16, wp=wp16, bp=bp16, mask=mask,
                id32=id32, id16=id16)


def _run(x, Wq, Wk, Wv, Wp, bp, trace=False):
    if "nc" not in _CACHE:
        _CACHE["nc"] = _build_nc()
    nc = _CACHE["nc"]

    consts = _host_prep(
        np.asarray(Wq), np.asarray(Wk), np.asarray(Wv),
        np.asarray(Wp), np.asarray(bp),
    )
    x = np.asarray(x)
    in_maps = []
    for c in range(NCORES):
        xs = np.ascontiguousarray(
            x[c * BC:(c + 1) * BC].reshape(BT, E), dtype=np.float32
        )
        in_maps.append({"xs": xs, **consts})

    res = run_bass_kernel_spmd(
        nc, in_maps, core_ids=list(range(NCORES)), trace=trace
    )
    y = np.concatenate(
        [res.results[c]["ys"].reshape(BC, T, E) for c in range(NCORES)], axis=0
    )
    return y.astype(np.float32), res


def kernel(x, Wq, Wk, Wv, Wp, bp):
    y, _ = _run(x, Wq, Wk, Wv, Wp, bp, trace=False)
    return y



# revision 2
# speedup vs baseline: 26.1857x; 26.1857x over previous
"""v5 variant. Trainium2 Bass kernel for nn_MultiHeadAttention (B=2048, T=32, E=1024, H=16).

v4: error-compensated fp8e4m3 DoubleRow matmuls for ALL four projections
(Q/K/V/out), fp16 attention core, XBAR DMA transposes, 2-stage software
pipeline (x-chain and Q/K/V of tile mt+1 prefetched between the attention
and output-projection phases of tile mt).

Compensation (per operand pair, dropping only the residual*residual term):
  x16 = f16(x); x8 = e4m3(x16); dx8 = e4m3(x16 - x8)
  W32 = 32*W;   w8 = e4m3(W32);  dw8 = e4m3(W32 - w8)
  x16 @ W32 via DoubleRow: 4 A-matmuls (x8.w8, K=256 each) +
  8 B-matmuls ((x8.dw8 + dx8.w8), K=128 real each) = 6N cycles/K=1024
  vs fp16's 8N. Same for the output projection with O residuals.
Scales: QT/KT psum = 32q/32k folded into exp scale; V' ones-col = 32
cancels; y psum = 32(O@Wp + bp), evac scale 1/32. Bias via fp8-DR
ones-(1,0) pair matmul.

Sharding: data-parallel over batch, 256 batches per core x 8 cores.
"""
import sys
import numpy as np

sys.path.insert(0, "/opt/trn_rl_repo")

import concourse.bass as bass  # noqa: E402
import concourse.bacc as bacc  # noqa: E402
import concourse.mybir as mybir  # noqa: E402
import concourse.tile as tile  # noqa: E402
from concourse.bass_utils import run_bass_kernel_spmd  # noqa: E402

B, T, E, H = 2048, 32, 1024, 16
HS = E // H            # 64
NCORES = 8
BC = B // NCORES       # 256 batches per core
BT = BC * T            # 8192 rows per core
P = 128
ET = E // P            # 8 e-tiles
MT = 512               # rows per macro-tile
NMT = BT // MT         # 16
NBT = MT // P          # 4 bt-tiles per macro-tile

F8 = mybir.dt.float8e4
F16 = mybir.dt.float16
F32 = mybir.dt.float32
AF = mybir.ActivationFunctionType
DR = mybir.MatmulPerfMode.DoubleRow

WS = 32.0
EXP_SCALE = (HS ** -0.5) / (WS * WS)

HEAD_GROUPS = ((0, 2, 4, 6), (8, 10, 12, 14), (1, 3, 5, 7), (9, 11, 13, 15))

_CACHE = {}


def _build_nc(nmt=NMT):
    nc = bacc.Bacc(trn_type="TRN2")

    x_d = nc.dram_tensor("xs", [nmt * MT, E], F32, kind="ExternalInput")
    y_d = nc.dram_tensor("ys", [nmt * MT, E], F32, kind="ExternalOutput")
    # compensated fp8 weights: [p, et, slot(2: dw8, w8), c]
    wq_d = nc.dram_tensor("wq", [P, ET * 2 * E], F8, kind="ExternalInput")
    wk_d = nc.dram_tensor("wk", [P, ET * 2 * E], F8, kind="ExternalInput")
    wv_d = nc.dram_tensor("wv", [P, ET * 2 * E], F8, kind="ExternalInput")
    wp_d = nc.dram_tensor("wp", [P, ET * 2 * E], F8, kind="ExternalInput")
    bp_d = nc.dram_tensor("bp", [1, 2 * E], F8, kind="ExternalInput")
    mask_d = nc.dram_tensor("mask", [P, 512], F16, kind="ExternalInput")

    with tile.TileContext(nc) as tc:
        with (
            tc.tile_pool(name="const", bufs=1) as cpool,
            tc.tile_pool(name="xin", bufs=4) as xpool,
            tc.tile_pool(name="xt16", bufs=2) as xtpool,
            tc.tile_pool(name="xd", bufs=2) as xdpool,
            tc.tile_pool(name="qt", bufs=9) as qtpool,
            tc.tile_pool(name="kt", bufs=9) as ktpool,
            tc.tile_pool(name="vp", bufs=6) as vppool,
            tc.tile_pool(name="eb", bufs=5) as ebpool,
            tc.tile_pool(name="o16", bufs=4) as opool,
            tc.tile_pool(name="ot16", bufs=5) as otpool,
            tc.tile_pool(name="od", bufs=5) as odpool,
            tc.tile_pool(name="rc", bufs=4) as rcpool,
            tc.tile_pool(name="yo", bufs=4) as ypool,
            tc.tile_pool(name="ps_mm", bufs=4, space="PSUM") as psmm,
            tc.tile_pool(name="ps_s", bufs=2, space="PSUM") as pss,
            tc.tile_pool(name="ps_u", bufs=2, space="PSUM") as psu,
        ):
            wq = cpool.tile([P, ET * 2 * E], F8)
            wk = cpool.tile([P, ET * 2 * E], F8)
            wv = cpool.tile([P, ET * 2 * E], F8)
            wp = cpool.tile([P, ET * 2 * E], F8)
            bpt = cpool.tile([1, 2 * E], F8)   # [slot(2: bp8, junk), e] pairs
            mask = cpool.tile([P, 512], F16)
            ones8 = cpool.tile([1, 2 * P], F8)  # slot0 = 1, slot1 = 0

            def load_weights():
                CH = ET * 2 * E // 4
                for c in range(4):
                    sl = slice(CH * c, CH * (c + 1))
                    nc.gpsimd.dma_start(wq[:, sl], wq_d[:, sl])
                    nc.gpsimd.dma_start(wk[:, sl], wk_d[:, sl])
                    nc.scalar.dma_start(wv[:, sl], wv_d[:, sl])
                    nc.scalar.dma_start(wp[:, sl], wp_d[:, sl])
                nc.gpsimd.dma_start(bpt[:], bp_d[:])
                nc.gpsimd.dma_start(mask[:], mask_d[:])
                nc.vector.memset(ones8[:, 0:P], 1.0)
                nc.vector.memset(ones8[:, P:2 * P], 0.0)

            wqv = wq.rearrange("p (et s e) -> p et s e", s=2, e=E)
            wkv = wk.rearrange("p (et s e) -> p et s e", s=2, e=E)
            wvv = wv.rearrange("p (et s e) -> p et s e", s=2, e=E)
            wpv = wp.rearrange("p (et s e) -> p et s e", s=2, e=E)
            onev = ones8.rearrange("p (s c) -> p s c", s=2)
            bpv = bpt.rearrange("p (s e) -> p s e", s=2)

            x_v = x_d.rearrange("(m b p) e -> m b p e", b=NBT, p=P)
            y_v = y_d.rearrange("(m b p) e -> m b p e", b=NBT, p=P)

            def pre(mt):
                """x load -> f16 -> XBAR transpose -> fp8 + residual."""
                x16s = []
                for b in range(NBT):
                    xt_in = xpool.tile([P, E], F32, tag="x")
                    nc.sync.dma_start(xt_in[:], x_v[mt, b])
                    x16 = xpool.tile([P, E], F16, tag="x16")
                    nc.vector.tensor_copy(x16[:], xt_in[:])
                    x16s.append(x16)
                xt16 = xtpool.tile([P, ET * 512], F16, tag="xt16")
                xtv = xt16.rearrange("p (et c) -> p et c", c=512)
                for b in range(NBT):
                    nc.sync.dma_start_transpose(
                        xtv[:, :, P * b:P * (b + 1)], x16s[b][:]
                    )
                xd = xdpool.tile([P, 2 * ET * 512], F8, tag="xd")
                for h in range(4):
                    sl = slice(1024 * h, 1024 * (h + 1))
                    nc.gpsimd.tensor_copy(xd[:, sl], xt16[:, sl])
                    nc.gpsimd.tensor_sub(
                        xd[:, 4096 + 1024 * h:4096 + 1024 * (h + 1)],
                        xt16[:, sl], xd[:, sl],
                    )
                return xd.rearrange("p (s et c) -> p s et c", s=2, c=512)

            def proj(xdv):
                """Q/K/V projections from the fp8(+residual) transposed x."""
                qts, kts = [], []
                for w8v, pool, outl in ((wqv, qtpool, qts), (wkv, ktpool, kts)):
                    for ht in range(ET):
                        pq = psmm.tile([P, 512], F32, tag="ps_mm")
                        cs = slice(P * ht, P * (ht + 1))
                        for e2 in range(4):
                            nc.tensor.matmul(          # A: x8.w8 K=256
                                pq[:],
                                w8v[:, 2 * e2:2 * e2 + 2, 1, cs],
                                xdv[:, 0, 2 * e2:2 * e2 + 2, :],
                                start=(e2 == 0), stop=False, perf_mode=DR,
                            )
                        for et in range(ET):
                            nc.tensor.matmul(          # B: x8.dw8 + dx8.w8
                                pq[:],
                                w8v[:, et, :, cs],
                                xdv[:, :, et, :],
                                start=False, stop=(et == ET - 1), perf_mode=DR,
                            )
                        sb = pool.tile([P, 512], F16)
                        nc.vector.tensor_copy(sb[:], pq[:])
                        outl.append(sb)
                vps = []
                for b in range(NBT):
                    vp_t = vppool.tile([P, H * (HS + 1)], F16, tag="vp")
                    nc.vector.memset(
                        vp_t.rearrange("p (h c) -> p h c", c=HS + 1)[:, :, HS:HS + 1],
                        WS,
                    )
                    bs = slice(P * b, P * (b + 1))
                    for h2 in range(2):
                        pv = psmm.tile([P, 512], F32, tag="ps_mm")
                        ms = slice(512 * h2, 512 * (h2 + 1))
                        for e2 in range(4):
                            nc.tensor.matmul(          # A
                                pv[:],
                                xdv[:, 0, 2 * e2:2 * e2 + 2, bs],
                                wvv[:, 2 * e2:2 * e2 + 2, 1, ms],
                                start=(e2 == 0), stop=False, perf_mode=DR,
                            )
                        for et in range(ET):
                            nc.tensor.matmul(          # B
                                pv[:],
                                xdv[:, :, et, bs],
                                wvv[:, et, :, ms],
                                start=False, stop=(et == ET - 1), perf_mode=DR,
                            )
                        dst = vp_t[:, 8 * (HS + 1) * h2:8 * (HS + 1) * (h2 + 1)]
                        nc.scalar.activation(
                            dst.rearrange("p (h c) -> p h c", c=HS + 1)[:, :, 0:HS],
                            pv.rearrange("p (h c) -> p h c", c=HS)[:],
                            AF.Copy,
                        )
                    vps.append(vp_t)
                return qts, kts, vps

            def attention(qts, kts, vps):
                odvs = []
                for b in range(NBT):
                    o_t = opool.tile([P, E], F16, tag="o")
                    for hq in range(4):
                        heads = HEAD_GROUPS[hq]
                        ps_s = pss.tile([P, 512], F32, tag="ps_s")
                        for hh in range(4):
                            h = heads[hh]
                            ht, hp = divmod(h, 2)
                            rs = slice(64 * hp, 64 * (hp + 1))
                            cs = slice(P * b, P * (b + 1))
                            nc.tensor.matmul(
                                ps_s[:, P * hh:P * (hh + 1)],
                                kts[ht][rs, cs],
                                qts[ht][rs, cs],
                                start=True, stop=True,
                            )
                        ebt = ebpool.tile([P, 512], F16, tag="eb")
                        nc.scalar.activation(ebt[:], ps_s[:], AF.Exp,
                                             scale=EXP_SCALE)
                        nc.gpsimd.tensor_mul(ebt[:], ebt[:], mask[:])
                        pu = psu.tile([P, 4 * (HS + 1)], F32, tag="ps_u")
                        for hh in range(4):
                            h = heads[hh]
                            nc.tensor.matmul(
                                pu[:, (HS + 1) * hh:(HS + 1) * (hh + 1)],
                                ebt[:, P * hh:P * (hh + 1)],
                                vps[b][:, (HS + 1) * h:(HS + 1) * (h + 1)],
                                start=True, stop=True,
                            )
                        rc_t = rcpool.tile([P, 4], F32, tag="rc")
                        nc.vector.reciprocal(
                            rc_t[:],
                            pu.rearrange("p (h c) -> p h c", c=HS + 1)[:, :, HS:HS + 1],
                        )
                        for hh in range(4):
                            h = heads[hh]
                            dst = o_t[:, HS * h:HS * (h + 1)]
                            src = pu[:, (HS + 1) * hh:(HS + 1) * hh + HS]
                            sc = rc_t[:, hh:hh + 1]
                            if hh % 2 == 0:
                                nc.scalar.activation(dst, src, AF.Copy, scale=sc)
                            else:
                                nc.vector.tensor_scalar_mul(dst, src, sc)
                    odvs.append(ot_block(o_t))
                return odvs

            def ot_block(o_t):
                """Per-block OT16 via XBAR DMA transpose, then fp8+residual."""
                ot16 = otpool.tile([P, ET * P], F16, tag="ot16")
                nc.sync.dma_start_transpose(
                    ot16.rearrange("p (et c) -> p et c", c=P)[:, :, :], o_t[:]
                )
                od = odpool.tile([P, 2 * ET * P], F8, tag="od")
                nc.gpsimd.tensor_copy(od[:, 0:1024], ot16[:])
                nc.gpsimd.tensor_sub(od[:, 1024:2048], ot16[:], od[:, 0:1024])
                return od.rearrange("p (s et c) -> p s et c", s=2, c=P)

            def yproj(mt, odvs):
                for b in range(NBT):
                    odv = odvs[b]
                    for e2 in range(2):
                        py = psmm.tile([P, 512], F32, tag="ps_mm")
                        ms = slice(512 * e2, 512 * (e2 + 1))
                        nc.tensor.matmul(      # bias: (1,0) ones pair . bp8
                            py[:],
                            onev[:, :, :],
                            bpv[:, :, ms],
                            start=True, stop=False, perf_mode=DR,
                            skip_group_check=True,
                        )
                        for e4 in range(4):
                            nc.tensor.matmul(  # A
                                py[:],
                                odv[:, 0, 2 * e4:2 * e4 + 2, :],
                                wpv[:, 2 * e4:2 * e4 + 2, 1, ms],
                                start=False, stop=False, perf_mode=DR,
                                skip_group_check=True,
                            )
                        for et in range(ET):
                            nc.tensor.matmul(  # B
                                py[:],
                                odv[:, :, et, :],
                                wpv[:, et, :, ms],
                                start=False, stop=(et == ET - 1), perf_mode=DR,
                                skip_group_check=True,
                            )
                        y_t = ypool.tile([P, 512], F32, tag="y")
                        nc.vector.tensor_scalar_mul(y_t[:], py[:], 1.0 / WS)
                        nc.sync.dma_start(y_v[mt, b][:, ms], y_t[:])

            # ---- 2-stage pipeline ----
            xdv = pre(0)
            load_weights()
            qts, kts, vps = proj(xdv)
            for mt in range(nmt):
                odvs = attention(qts, kts, vps)
                if mt + 1 < nmt:
                    xdv = pre(mt + 1)
                    nqts, nkts, nvps = proj(xdv)
                yproj(mt, odvs)
                if mt + 1 < nmt:
                    qts, kts, vps = nqts, nkts, nvps

    nc.compile()
    return nc


def _host_prep(Wq, Wk, Wv, Wp, bp):
    f8np = mybir.dt.np(F8)

    def cat(w):  # [H, E, HS] -> [E, E]
        return np.ascontiguousarray(w.transpose(1, 0, 2).reshape(E, E))

    def comp_layout(W):  # [E, E] f32 -> [p, et*2*E] fp8 (slot0=dw8, slot1=w8)
        W32 = (W * WS).astype(np.float32)
        w8 = np.clip(W32, -240, 240).astype(f8np)
        dw = W32 - w8.astype(np.float32)
        dw8 = np.clip(dw, -240, 240).astype(f8np)
        w8c = w8.reshape(ET, P, E).transpose(1, 0, 2)
        dw8c = dw8.reshape(ET, P, E).transpose(1, 0, 2)
        st = np.stack([dw8c, w8c], axis=2)  # [p, et, 2, c]
        return np.ascontiguousarray(st.reshape(P, ET * 2 * E))

    wq8 = comp_layout(cat(Wq))
    wk8 = comp_layout(cat(Wk))
    wv8 = comp_layout(cat(Wv))
    wp8 = comp_layout(Wp.astype(np.float32))
    bp8 = np.zeros((1, 2 * E), dtype=f8np)
    bp8[0, 0:E] = np.clip(bp * WS, -240, 240).astype(f8np)

    m = np.zeros((P, P), dtype=np.float16)
    trilT = np.tril(np.ones((T, T))).T.astype(np.float16)  # [s,t], s<=t
    for i in range(4):
        m[T * i:T * (i + 1), T * i:T * (i + 1)] = trilT
    mask = np.ascontiguousarray(np.tile(m, (1, 4)))

    return dict(wq=wq8, wk=wk8, wv=wv8, wp=wp8, bp=bp8, mask=mask)


def _run(x, Wq, Wk, Wv, Wp, bp, trace=False):
    if "nc" not in _CACHE:
        _CACHE["nc"] = _build_nc()
    nc = _CACHE["nc"]

    consts = _host_prep(
        np.asarray(Wq), np.asarray(Wk), np.asarray(Wv),
        np.asarray(Wp), np.asarray(bp),
    )
    x = np.asarray(x)
    in_maps = []
    for c in range(NCORES):
        xs = np.ascontiguousarray(
            x[c * BC:(c + 1) * BC].reshape(BT, E), dtype=np.float32
        )
        in_maps.append({"xs": xs, **consts})

    res = run_bass_kernel_spmd(
        nc, in_maps, core_ids=list(range(NCORES)), trace=trace
    )
    y = np.concatenate(
        [res.results[c]["ys"].reshape(BC, T, E) for c in range(NCORES)], axis=0
    )
    return y.astype(np.float32), res


def kernel(x, Wq, Wk, Wv, Wp, bp):
    y, _ = _run(x, Wq, Wk, Wv, Wp, bp, trace=False)
    return y
